# revision 4
# baseline (speedup 1.0000x reference)
"""Trainium2 Bass kernel for LongNet-style dilated attention.

Module config (hardcoded): x [4, 8192, 2048] f32, d_model=2048, 16 heads,
head_dim=128, segment=512, dilation=2.

Math per (batch, segment, head):
  g = x[b, seg, offset_h::2, h*128:(h+1)*128]          # [256, 128]
  A = softmax(g @ g.T / sqrt(128))                      # [256, 256]
  out[b, seg, offset_h::2, h*128:(h+1)*128] = A @ g     # rest stays 0

Sharding: 64 segments (4 batches x 16 segs) split 8-per-core across the
8 NeuronCores; segments are fully independent (no collectives).

Per-core kernel layout notes:
  - tokens of one parity are loaded token-major [128t, 2, 2048d] (2 token
    blocks in the free dim), 2MB per DMA.
  - gT (d-major) produced by PE transposes; S = gT.T @ gT computed in
    float32r (1 cycle/row at N=256).
  - exp on ScalarE with fused accum_out rowsums; softmax normalization is
    folded into the PSUM->SBUF copy of the output (tensor_scalar mult by
    reciprocal rowsum). S and exp(S) are symmetric, so exp(S) tiles serve
    directly as the transposed stationary operand of A @ g -- no second
    transpose pass.
  - only dilated positions are written back (strided DMA); the harness's
    output buffers are zero-initialized, giving the zeros elsewhere.
"""

import numpy as np

import concourse.bacc as bacc
import concourse.bass as bass
import concourse.tile as tile
from concourse import mybir
from concourse.bass_utils import run_bass_kernel_spmd
from concourse.masks import make_identity

N_CORES = 8
B = 4
N_TOK = 8192
D = 2048
H = 16
HD = 128
SEG = 512
SDIL = 256  # dilated tokens per segment per head (SEG / dilation)
SCALE = 1.0 / float(np.sqrt(HD))

SEGS_TOTAL = (B * N_TOK) // SEG  # 64
SEGS_PER_CORE = SEGS_TOTAL // N_CORES  # 8

FP32 = mybir.dt.float32
FP32R = mybir.dt.float32r
EXP = mybir.ActivationFunctionType.Exp


def build_nc(n_segs=SEGS_PER_CORE, s_dtype=FP32R, o_dtype=FP32):
    """Build the per-core Bass program for n_segs segments."""
    nc = bacc.Bacc(
        "TRN2", target_bir_lowering=False, debug=False, num_devices=N_CORES
    )
    ntok = n_segs * SEG
    x = nc.dram_tensor("x", [ntok, D], FP32, kind="ExternalInput").ap()
    out = nc.dram_tensor("out", [ntok, D], FP32, kind="ExternalOutput").ap()

    # row n = s*512 + t*2 + u  (u = parity, t = dilated index)
    xv = x.rearrange("(s t u) d -> s u t d", u=2, t=SDIL)
    # col d = hh*256 + uu*128 + c  (head h = 2*hh + uu)
    ov = out.rearrange(
        "(s t u) (hh uu c) -> s u t hh uu c", t=SDIL, u=2, uu=2, c=HD
    )

    with tile.TileContext(nc) as tc:
        with (
            tc.tile_pool(name="xp", bufs=3) as xp_pool,
            tc.tile_pool(name="gt", bufs=3) as gt_pool,
            tc.tile_pool(name="ee", bufs=3) as e_pool,
            tc.tile_pool(name="small", bufs=3) as small_pool,
            tc.tile_pool(name="stage", bufs=3) as stage_pool,
            tc.tile_pool(name="const", bufs=1) as const_pool,
            tc.tile_pool(name="gtps", bufs=2, space="PSUM") as gtps_pool,
            tc.tile_pool(name="sps", bufs=2, space="PSUM") as sps_pool,
            tc.tile_pool(name="ops", bufs=2, space="PSUM") as ops_pool,
        ):
            ident = const_pool.tile([128, 128], FP32)
            make_identity(nc, ident)

            for s in range(n_segs):
                for u in range(2):
                    # xp[t, i, d] = x[s*512 + (i*128 + t)*2 + u, d]
                    xp = xp_pool.tile([128, 2, D], FP32, tag="xp")
                    nc.sync.dma_start(
                        out=xp,
                        in_=xv[s, u].rearrange("(i t) d -> t i d", i=2),
                    )
                    rs = small_pool.tile([128, 8, 2], FP32, tag="rs")
                    rcp = small_pool.tile([128, 8, 2], FP32, tag="rcp")
                    stage = stage_pool.tile([128, 2, 8, HD], FP32, tag="stage")

                    for hh in range(8):
                        h = 2 * hh + u
                        cs = slice(h * HD, (h + 1) * HD)

                        # gT[d, k] for the 256 dilated tokens of this head
                        gt_ps = gtps_pool.tile([128, 256], FP32)
                        nc.tensor.transpose(gt_ps[:, 0:128], xp[:, 0, cs], ident)
                        nc.tensor.transpose(gt_ps[:, 128:256], xp[:, 1, cs], ident)
                        gt = gt_pool.tile([128, 256], s_dtype, tag="gt")
                        if hh % 2 == 0:
                            nc.scalar.copy(gt, gt_ps)
                        else:
                            nc.vector.tensor_copy(gt, gt_ps)

                        # S[q, k] = gT.T @ gT (scaled later inside exp)
                        s_ps = sps_pool.tile([128, 512], FP32)
                        gtm = gt
                        nc.tensor.matmul(
                            s_ps[:, 0:256], gtm[:, 0:128], gtm, start=True, stop=True
                        )
                        nc.tensor.matmul(
                            s_ps[:, 256:512], gtm[:, 128:256], gtm, start=True, stop=True
                        )

                        # E = exp(S * scale); rowsums fused via accum_out
                        e = e_pool.tile([128, 512], FP32, tag="ee")
                        nc.scalar.activation(
                            e[:, 0:256], s_ps[:, 0:256], EXP,
                            scale=SCALE, accum_out=rs[:, hh, 0:1],
                        )
                        nc.scalar.activation(
                            e[:, 256:512], s_ps[:, 256:512], EXP,
                            scale=SCALE, accum_out=rs[:, hh, 1:2],
                        )
                        nc.vector.reciprocal(rcp[:, hh, :], rs[:, hh, :])

                        # out[q, d] = sum_k E[k, q] * g[k, d]  (E symmetric)
                        o_ps = ops_pool.tile([128, 256], FP32)
                        em = e.bitcast(o_dtype) if o_dtype != FP32 else e
                        xm = xp.bitcast(o_dtype) if o_dtype != FP32 else xp
                        # (k-chunk, q-chunk) lhsT slices of E; rhs = raw x slice
                        nc.tensor.matmul(
                            o_ps[:, 0:128], em[:, 0:128], xm[:, 0, cs],
                            start=True, stop=False,
                        )
                        nc.tensor.matmul(
                            o_ps[:, 0:128], em[:, 256:384], xm[:, 1, cs],
                            start=False, stop=True,
                        )
                        nc.tensor.matmul(
                            o_ps[:, 128:256], em[:, 128:256], xm[:, 0, cs],
                            start=True, stop=False,
                        )
                        nc.tensor.matmul(
                            o_ps[:, 128:256], em[:, 384:512], xm[:, 1, cs],
                            start=False, stop=True,
                        )

                        # normalize while copying PSUM->SBUF staging
                        for qc in range(2):
                            nc.vector.tensor_scalar_mul(
                                stage[:, qc, hh, :],
                                o_ps[:, qc * 128:(qc + 1) * 128],
                                rcp[:, hh, qc:qc + 1],
                            )

                    for qc in range(2):
                        nc.sync.dma_start(
                            out=ov[s, u, qc * 128:(qc + 1) * 128, :, u, :],
                            in_=stage[:, qc],
                        )

    nc.compile()
    return nc


_NC_CACHE = {}


def _get_nc():
    key = "full"
    if key not in _NC_CACHE:
        _NC_CACHE[key] = build_nc()
    return _NC_CACHE[key]


def make_in_maps(x: np.ndarray):
    xs = np.ascontiguousarray(x).reshape(SEGS_TOTAL, SEG, D)
    in_maps = []
    for c in range(N_CORES):
        chunk = xs[c * SEGS_PER_CORE:(c + 1) * SEGS_PER_CORE]
        in_maps.append(
            {"x": np.ascontiguousarray(chunk).reshape(SEGS_PER_CORE * SEG, D)}
        )
    return in_maps


def gather_out(results) -> np.ndarray:
    outs = [results[c]["out"] for c in range(N_CORES)]
    return np.concatenate(outs, axis=0).reshape(B, N_TOK, D)


def kernel(x: np.ndarray) -> np.ndarray:
    assert x.shape == (B, N_TOK, D) and x.dtype == np.float32
    nc = _get_nc()
    res = run_bass_kernel_spmd(nc, make_in_maps(x), list(range(N_CORES)))
    return gather_out(res.results)


# revision 10
# speedup vs baseline: 1.0106x; 1.0106x over previous
"""Trainium2 Bass kernel for LongNet-style dilated attention.

Module config (hardcoded): x [4, 8192, 2048] f32, d_model=2048, 16 heads,
head_dim=128, segment=512, dilation=2.

Math per (batch, segment, head):
  g = x[b, seg, offset_h::2, h*128:(h+1)*128]          # [256, 128]
  A = softmax(g @ g.T / sqrt(128))                      # [256, 256]
  out[b, seg, offset_h::2, h*128:(h+1)*128] = A @ g     # rest stays 0

Sharding: 64 segments (4 batches x 16 segs) split 8-per-core across the
8 NeuronCores; segments are fully independent (no collectives).

Per-core kernel layout notes:
  - tokens of one parity are loaded token-major [128t, 2, 2048d] (2 token
    blocks in the free dim), 2MB per DMA.
  - gT (d-major) produced by PE transposes; S = gT.T @ gT computed in
    float32r (1 cycle/row at N=256).
  - exp on ScalarE with fused accum_out rowsums; softmax normalization is
    folded into the PSUM->SBUF copy of the output (tensor_scalar mult by
    reciprocal rowsum). S and exp(S) are symmetric, so exp(S) tiles serve
    directly as the transposed stationary operand of A @ g -- no second
    transpose pass.
  - only dilated positions are written back (strided DMA); the harness's
    output buffers are zero-initialized, giving the zeros elsewhere.
"""

import numpy as np

import concourse.bacc as bacc
import concourse.bass as bass
import concourse.tile as tile
from concourse import mybir
from concourse.bass_utils import run_bass_kernel_spmd
from concourse.masks import make_identity

N_CORES = 8
B = 4
N_TOK = 8192
D = 2048
H = 16
HD = 128
SEG = 512
SDIL = 256  # dilated tokens per segment per head (SEG / dilation)
SCALE = 1.0 / float(np.sqrt(HD))

SEGS_TOTAL = (B * N_TOK) // SEG  # 64
SEGS_PER_CORE = SEGS_TOTAL // N_CORES  # 8

FP32 = mybir.dt.float32
FP32R = mybir.dt.float32r
BF16 = mybir.dt.bfloat16
EXP = mybir.ActivationFunctionType.Exp


def build_nc(n_segs=SEGS_PER_CORE, s_dtype=FP32R, o_dtype=BF16):
    """Build the per-core Bass program for n_segs segments."""
    nc = bacc.Bacc(
        "TRN2", target_bir_lowering=False, debug=False, num_devices=N_CORES
    )
    ntok = n_segs * SEG
    x = nc.dram_tensor("x", [ntok, D], FP32, kind="ExternalInput").ap()
    out = nc.dram_tensor("out", [ntok, D], FP32, kind="ExternalOutput").ap()

    # row n = s*512 + t*2 + u  (u = parity, t = dilated index)
    xv = x.rearrange("(s t u) d -> s u t d", u=2, t=SDIL)
    # col d = hh*256 + uu*128 + c  (head h = 2*hh + uu)
    ov = out.rearrange(
        "(s t u) (hh uu c) -> s u t hh uu c", t=SDIL, u=2, uu=2, c=HD
    )

    with tile.TileContext(nc) as tc:
        with (
            tc.tile_pool(name="xp", bufs=3) as xp_pool,
            tc.tile_pool(name="gt", bufs=3) as gt_pool,
            tc.tile_pool(name="ee", bufs=3) as e_pool,
            tc.tile_pool(name="small", bufs=3) as small_pool,
            tc.tile_pool(name="stage", bufs=3) as stage_pool,
            tc.tile_pool(name="const", bufs=1) as const_pool,
            tc.tile_pool(name="gtps", bufs=2, space="PSUM") as gtps_pool,
            tc.tile_pool(name="sps", bufs=3, space="PSUM") as sps_pool,
            tc.tile_pool(name="ops", bufs=3, space="PSUM") as ops_pool,
        ):
            ident = const_pool.tile([128, 128], FP32)
            make_identity(nc, ident)

            for s in range(n_segs):
                for u in range(2):
                    # xp[t, i, d] = x[s*512 + (i*128 + t)*2 + u, d]
                    xp = xp_pool.tile([128, 2, D], FP32, tag="xp")
                    nc.sync.dma_start(
                        out=xp,
                        in_=xv[s, u].rearrange("(i t) d -> t i d", i=2),
                    )
                    if o_dtype == BF16:
                        # bf16 shadow of xp for the A@g matmul rhs (GPSIMD
                        # is otherwise idle; the cast rides on it)
                        xb = xp_pool.tile([128, 2, D], BF16, tag="xb")
                        nc.gpsimd.tensor_copy(xb, xp)
                    else:
                        xb = xp
                    rs = small_pool.tile([128, 8, 2], FP32, tag="rs")
                    rcp = small_pool.tile([128, 8, 2], FP32, tag="rcp")
                    stage = stage_pool.tile([128, 2, 8, HD], FP32, tag="stage")

                    for hh in range(8):
                        h = 2 * hh + u
                        cs = slice(h * HD, (h + 1) * HD)

                        # gT[d, k] for the 256 dilated tokens of this head
                        gt_ps = gtps_pool.tile([128, 256], FP32)
                        nc.tensor.transpose(gt_ps[:, 0:128], xp[:, 0, cs], ident)
                        nc.tensor.transpose(gt_ps[:, 128:256], xp[:, 1, cs], ident)
                        gt = gt_pool.tile([128, 256], s_dtype, tag="gt")
                        if hh % 2 == 0:
                            nc.scalar.copy(gt, gt_ps)
                        else:
                            nc.vector.tensor_copy(gt, gt_ps)

                        # S[q, k] = gT.T @ gT (scaled later inside exp)
                        s_ps = sps_pool.tile([128, 512], FP32)
                        gtm = gt
                        nc.tensor.matmul(
                            s_ps[:, 0:256], gtm[:, 0:128], gtm, start=True, stop=True
                        )
                        nc.tensor.matmul(
                            s_ps[:, 256:512], gtm[:, 128:256], gtm, start=True, stop=True
                        )

                        # E = exp(S * scale); rowsums fused via accum_out
                        e = e_pool.tile([128, 512], o_dtype, tag="ee")
                        nc.scalar.activation(
                            e[:, 0:256], s_ps[:, 0:256], EXP,
                            scale=SCALE, accum_out=rs[:, hh, 0:1],
                        )
                        nc.scalar.activation(
                            e[:, 256:512], s_ps[:, 256:512], EXP,
                            scale=SCALE, accum_out=rs[:, hh, 1:2],
                        )
                        nc.vector.reciprocal(rcp[:, hh, :], rs[:, hh, :])

                        # out[q, d] = sum_k E[k, q] * g[k, d]  (E symmetric)
                        o_ps = ops_pool.tile([128, 256], FP32)
                        em = e
                        xm = xb
                        # (k-chunk, q-chunk) lhsT slices of E; rhs = raw x slice
                        nc.tensor.matmul(
                            o_ps[:, 0:128], em[:, 0:128], xm[:, 0, cs],
                            start=True, stop=False,
                        )
                        nc.tensor.matmul(
                            o_ps[:, 0:128], em[:, 256:384], xm[:, 1, cs],
                            start=False, stop=True,
                        )
                        nc.tensor.matmul(
                            o_ps[:, 128:256], em[:, 128:256], xm[:, 0, cs],
                            start=True, stop=False,
                        )
                        nc.tensor.matmul(
                            o_ps[:, 128:256], em[:, 384:512], xm[:, 1, cs],
                            start=False, stop=True,
                        )

                        # normalize while copying PSUM->SBUF staging
                        for qc in range(2):
                            nc.vector.tensor_scalar_mul(
                                stage[:, qc, hh, :],
                                o_ps[:, qc * 128:(qc + 1) * 128],
                                rcp[:, hh, qc:qc + 1],
                            )

                    for qc in range(2):
                        nc.sync.dma_start(
                            out=ov[s, u, qc * 128:(qc + 1) * 128, :, u, :],
                            in_=stage[:, qc],
                        )

    nc.compile()
    return nc


_NC_CACHE = {}


def _get_nc():
    key = "full"
    if key not in _NC_CACHE:
        _NC_CACHE[key] = build_nc()
    return _NC_CACHE[key]


def make_in_maps(x: np.ndarray):
    xs = np.ascontiguousarray(x).reshape(SEGS_TOTAL, SEG, D)
    in_maps = []
    for c in range(N_CORES):
        chunk = xs[c * SEGS_PER_CORE:(c + 1) * SEGS_PER_CORE]
        in_maps.append(
            {"x": np.ascontiguousarray(chunk).reshape(SEGS_PER_CORE * SEG, D)}
        )
    return in_maps


def gather_out(results) -> np.ndarray:
    outs = [results[c]["out"] for c in range(N_CORES)]
    return np.concatenate(outs, axis=0).reshape(B, N_TOK, D)


def kernel(x: np.ndarray) -> np.ndarray:
    assert x.shape == (B, N_TOK, D) and x.dtype == np.float32
    nc = _get_nc()
    res = run_bass_kernel_spmd(nc, make_in_maps(x), list(range(N_CORES)))
    return gather_out(res.results)


# revision 13
# speedup vs baseline: 1.3627x; 1.3484x over previous
"""Trainium2 Bass kernel for LongNet-style dilated attention.

Module config (hardcoded): x [4, 8192, 2048] f32, d_model=2048, 16 heads,
head_dim=128, segment=512, dilation=2.

Math per (batch, segment, head):
  g = x[b, seg, offset_h::2, h*128:(h+1)*128]          # [256, 128]
  A = softmax(g @ g.T / sqrt(128))                      # [256, 256]
  out[b, seg, offset_h::2, h*128:(h+1)*128] = A @ g     # rest stays 0

Sharding: 64 segments (4 batches x 16 segs) split 8-per-core across the
8 NeuronCores; segments are fully independent (no collectives).

Per-core kernel layout notes:
  - tokens of one parity are loaded token-major [128t, 2, 2048d] (2 token
    blocks in the free dim), 2MB per DMA.
  - gT (d-major) produced by PE transposes; S = gT.T @ gT computed in
    float32r (1 cycle/row at N=256).
  - exp on ScalarE with fused accum_out rowsums; softmax normalization is
    folded into the PSUM->SBUF copy of the output (tensor_scalar mult by
    reciprocal rowsum). S and exp(S) are symmetric, so exp(S) tiles serve
    directly as the transposed stationary operand of A @ g -- no second
    transpose pass.
  - only dilated positions are written back (strided DMA); the harness's
    output buffers are zero-initialized, giving the zeros elsewhere.
"""

import numpy as np

import concourse.bacc as bacc
import concourse.bass as bass
import concourse.tile as tile
from concourse import mybir
from concourse.bass_utils import run_bass_kernel_spmd
from concourse.masks import make_identity

N_CORES = 8
B = 4
N_TOK = 8192
D = 2048
H = 16
HD = 128
SEG = 512
SDIL = 256  # dilated tokens per segment per head (SEG / dilation)
SCALE = 1.0 / float(np.sqrt(HD))

SEGS_TOTAL = (B * N_TOK) // SEG  # 64
SEGS_PER_CORE = SEGS_TOTAL // N_CORES  # 8

FP32 = mybir.dt.float32
FP32R = mybir.dt.float32r
BF16 = mybir.dt.bfloat16
EXP = mybir.ActivationFunctionType.Exp


def build_nc(n_segs=SEGS_PER_CORE, s_dtype=FP32R, o_dtype=BF16):
    """Build the per-core Bass program for n_segs segments."""
    nc = bacc.Bacc(
        "TRN2", target_bir_lowering=False, debug=False, num_devices=N_CORES
    )
    ntok = n_segs * SEG
    x = nc.dram_tensor("x", [ntok, D], FP32, kind="ExternalInput").ap()
    out = nc.dram_tensor("out", [ntok, D], FP32, kind="ExternalOutput").ap()

    # row n = s*512 + t*2 + u  (u = parity, t = dilated index)
    xv = x.rearrange("(s t u) d -> s u t d", u=2, t=SDIL)
    # col d = hh*256 + uu*128 + c  (head h = 2*hh + uu)
    ov = out.rearrange(
        "(s t u) (hh uu c) -> s u t hh uu c", t=SDIL, u=2, uu=2, c=HD
    )

    with tile.TileContext(nc) as tc:
        with (
            tc.tile_pool(name="xp", bufs=3) as xp_pool,
            tc.tile_pool(name="gt", bufs=3) as gt_pool,
            tc.tile_pool(name="ee", bufs=3) as e_pool,
            tc.tile_pool(name="small", bufs=3) as small_pool,
            tc.tile_pool(name="stage", bufs=3) as stage_pool,
            tc.tile_pool(name="const", bufs=1) as const_pool,
            tc.tile_pool(name="gtps", bufs=2, space="PSUM") as gtps_pool,
            tc.tile_pool(name="sps", bufs=2, space="PSUM") as sps_pool,
            tc.tile_pool(name="ops", bufs=4, space="PSUM") as ops_pool,
        ):
            ident = const_pool.tile([128, 128], FP32)
            make_identity(nc, ident)

            for s in range(n_segs):
                for u in range(2):
                    # xp[t, i, d] = x[s*512 + (i*128 + t)*2 + u, d]
                    xp = xp_pool.tile([128, 2, D], FP32, tag="xp")
                    nc.sync.dma_start(
                        out=xp,
                        in_=xv[s, u].rearrange("(i t) d -> t i d", i=2),
                    )
                    if o_dtype == BF16:
                        # bf16 shadow of xp for the A@g matmul rhs
                        xb = xp_pool.tile([128, 2, D], BF16, tag="xb")
                        nc.vector.tensor_copy(xb, xp)
                    else:
                        xb = xp
                    stage = stage_pool.tile([128, 2, 8, HD], FP32, tag="stage")

                    for hp in range(4):  # head pairs
                        e2 = e_pool.tile([128, 2, 512], o_dtype, tag="ee")
                        rs = small_pool.tile([128, 2, 2], FP32, tag="rs")
                        rcp = small_pool.tile([128, 2, 2], FP32, tag="rcp")
                        for j in range(2):
                            hh = 2 * hp + j
                            h = 2 * hh + u
                            cs = slice(h * HD, (h + 1) * HD)

                            # gT[d, k] for the 256 dilated tokens of this head
                            gt_ps = gtps_pool.tile([128, 256], FP32)
                            nc.tensor.transpose(gt_ps[:, 0:128], xp[:, 0, cs], ident)
                            nc.tensor.transpose(gt_ps[:, 128:256], xp[:, 1, cs], ident)
                            gt = gt_pool.tile([128, 256], s_dtype, tag="gt")
                            nc.scalar.copy(gt, gt_ps)

                            # S[q, k] = gT.T @ gT (scaled later inside exp)
                            s_ps = sps_pool.tile([128, 512], FP32)
                            nc.tensor.matmul(
                                s_ps[:, 0:256], gt[:, 0:128], gt,
                                start=True, stop=True,
                            )
                            nc.tensor.matmul(
                                s_ps[:, 256:512], gt[:, 128:256], gt,
                                start=True, stop=True,
                            )

                            # E = exp(S * scale), one batched instruction
                            e = e2[:, j, :]
                            nc.scalar.activation(e, s_ps, EXP, scale=SCALE)

                            # out[q, d] = sum_k E[k, q] * g[k, d]  (E symmetric)
                            o_ps = ops_pool.tile([128, 256], FP32)
                            nc.tensor.matmul(
                                o_ps[:, 0:128], e[:, 0:128], xb[:, 0, cs],
                                start=True, stop=False,
                            )
                            nc.tensor.matmul(
                                o_ps[:, 0:128], e[:, 256:384], xb[:, 1, cs],
                                start=False, stop=True,
                            )
                            nc.tensor.matmul(
                                o_ps[:, 128:256], e[:, 128:256], xb[:, 0, cs],
                                start=True, stop=False,
                            )
                            nc.tensor.matmul(
                                o_ps[:, 128:256], e[:, 384:512], xb[:, 1, cs],
                                start=False, stop=True,
                            )

                            if j == 0:
                                o_ps_pair = [o_ps]
                            else:
                                o_ps_pair.append(o_ps)

                        # rowsums for the pair: one batched DVE reduce + recip
                        nc.vector.reduce_sum(
                            rs,
                            e2.rearrange("p j (qc k) -> p (j qc) k", k=256),
                            axis=mybir.AxisListType.X,
                        )
                        nc.vector.reciprocal(rcp, rs)

                        # normalize while copying PSUM->SBUF staging
                        # (1 of 4 copies rides on ScalarE for engine balance)
                        for j in range(2):
                            hh = 2 * hp + j
                            for qc in range(2):
                                dst = stage[:, qc, hh, :]
                                src = o_ps_pair[j][:, qc * 128:(qc + 1) * 128]
                                sc = rcp[:, j, qc:qc + 1]
                                if j == 0 and qc == 0:
                                    nc.scalar.mul(dst, src, sc)
                                else:
                                    nc.vector.tensor_scalar_mul(dst, src, sc)

                    for qc in range(2):
                        nc.sync.dma_start(
                            out=ov[s, u, qc * 128:(qc + 1) * 128, :, u, :],
                            in_=stage[:, qc],
                        )

    nc.compile()
    return nc


_NC_CACHE = {}


def _get_nc():
    key = "full"
    if key not in _NC_CACHE:
        _NC_CACHE[key] = build_nc()
    return _NC_CACHE[key]


def make_in_maps(x: np.ndarray):
    xs = np.ascontiguousarray(x).reshape(SEGS_TOTAL, SEG, D)
    in_maps = []
    for c in range(N_CORES):
        chunk = xs[c * SEGS_PER_CORE:(c + 1) * SEGS_PER_CORE]
        in_maps.append(
            {"x": np.ascontiguousarray(chunk).reshape(SEGS_PER_CORE * SEG, D)}
        )
    return in_maps


def gather_out(results) -> np.ndarray:
    outs = [results[c]["out"] for c in range(N_CORES)]
    return np.concatenate(outs, axis=0).reshape(B, N_TOK, D)


def kernel(x: np.ndarray) -> np.ndarray:
    assert x.shape == (B, N_TOK, D) and x.dtype == np.float32
    nc = _get_nc()
    res = run_bass_kernel_spmd(nc, make_in_maps(x), list(range(N_CORES)))
    return gather_out(res.results)
